# revision 4
# baseline (speedup 1.0000x reference)
"""MoE AllGather token dispatcher (permute -> identity experts -> unpermute).

Strategy: shard tokens contiguously across the 8 cores (2048 tokens each).
Since the "experts" are identity, output_total[t] = hs[t] * sum(routed probs of t),
so each core only needs its own token rows:
  - read each of its token rows once (contiguous 1MB tiles),
  - indirect-DMA-scatter each row to its permuted positions (renumbered into a
    compact per-core buffer; host reassembles by global position),
  - multiply by the per-token prob sum and store its slice of output_total.
Per-core HBM traffic = 16MB read + 48MB write = I/O minimum for this op.
"""

import sys
import numpy as np

for _p in ("/opt/trn_rl_repo", "/root/.axon_site/_ro/trn_rl_repo"):
    if _p not in sys.path:
        sys.path.append(_p)

S, B, H = 4096, 4, 2048
E = 64
TOPK = 2
N = S * B               # 16384 tokens
M = N * TOPK            # 32768 permuted rows
NCORES = 8
TPC = N // NCORES       # 2048 tokens per core
P = 128
TILES = TPC // P        # 16 tiles of 128 tokens per core

_program_cache = {}


def _build_program(J, c_max, iters=1):
    """SPMD program: identical on all cores; per-core data differs only in inputs.

    iters > 1 repeats the whole pipeline (for slope-based timing only).
    """
    import concourse.bacc as bacc
    import concourse.bass as bass
    import concourse.mybir as mybir
    import concourse.tile as tile

    nc = bacc.Bacc()
    hs_chunk = nc.dram_tensor("hs_chunk", [TPC, H], mybir.dt.float32, kind="ExternalInput")
    dst_idx = nc.dram_tensor("dst_idx", [P, TILES * J], mybir.dt.int32, kind="ExternalInput")
    scale_in = nc.dram_tensor("scale_in", [P, TILES], mybir.dt.float32, kind="ExternalInput")
    perm_part = nc.dram_tensor("perm_part", [c_max, H], mybir.dt.float32, kind="ExternalOutput")
    out_scaled = nc.dram_tensor("out_scaled", [TPC, H], mybir.dt.float32, kind="ExternalOutput")

    with tile.TileContext(nc) as tc:
        with (
            tc.tile_pool(name="meta", bufs=1) as meta_pool,
            tc.tile_pool(name="io", bufs=4) as io_pool,
            tc.tile_pool(name="res", bufs=4) as res_pool,
        ):
            dst_t = meta_pool.tile([P, TILES * J], mybir.dt.int32)
            scale_t = meta_pool.tile([P, TILES], mybir.dt.float32)
            nc.sync.dma_start(out=dst_t[:], in_=dst_idx[:])
            nc.sync.dma_start(out=scale_t[:], in_=scale_in[:])
            for _it in range(iters):
                for i in range(TILES):
                    hs_t = io_pool.tile([P, H], mybir.dt.float32)
                    nc.sync.dma_start(out=hs_t[:], in_=hs_chunk[i * P:(i + 1) * P, :])
                    for j in range(J):
                        col = i * J + j
                        nc.gpsimd.indirect_dma_start(
                            out=perm_part[:],
                            out_offset=bass.IndirectOffsetOnAxis(
                                ap=dst_t[:, col:col + 1], axis=0
                            ),
                            in_=hs_t[:],
                            in_offset=None,
                            bounds_check=c_max - 1,
                            oob_is_err=False,
                        )
                    sc_t = res_pool.tile([P, H], mybir.dt.float32)
                    nc.vector.tensor_scalar_mul(sc_t[:], hs_t[:], scale_t[:, i:i + 1])
                    nc.scalar.dma_start(out=out_scaled[i * P:(i + 1) * P, :], in_=sc_t[:])

    nc.compile()
    return nc


def _plan(probs, routing_map):
    """Reproduce the reference's permutation order and build per-core scatter plans."""
    maskT = np.ascontiguousarray(routing_map.T).reshape(-1)       # [E*N]
    true_pos = np.flatnonzero(maskT)
    if true_pos.size >= M:
        sorted_idx = true_pos[:M]
    else:
        false_pos = np.flatnonzero(~maskT)
        sorted_idx = np.concatenate([true_pos, false_pos[: M - true_pos.size]])
    token_idx = sorted_idx % N                                    # [M] source token per row
    permuted_probs = np.ascontiguousarray(probs.T).reshape(-1)[sorted_idx]

    # per-token sum of routed probs (accumulated in f64, stored f32)
    scale = np.bincount(token_idx, weights=permuted_probs.astype(np.float64), minlength=N)
    scale = scale.astype(np.float32)

    # occurrence slot of each permuted row within its token (rows of one token
    # appear in increasing m; stable argsort preserves that order)
    order = np.argsort(token_idx, kind="stable")
    sorted_tok = token_idx[order]
    is_new = np.empty(M, dtype=bool)
    is_new[0] = True
    np.not_equal(sorted_tok[1:], sorted_tok[:-1], out=is_new[1:])
    group_id = np.cumsum(is_new) - 1
    group_start = np.flatnonzero(is_new)
    slot = np.empty(M, dtype=np.int64)
    slot[order] = np.arange(M) - group_start[group_id]
    J = int(slot.max()) + 1

    core_of = token_idx // TPC
    ms = [np.flatnonzero(core_of == k) for k in range(NCORES)]
    c_ks = [int(x.size) for x in ms]
    # round the compact buffer size up to a multiple of 2048 so the compiled
    # program (and the on-disk NEFF cache key) is stable across inputs
    c_max = max(max(c_ks), 1)
    c_max = (c_max + 2047) // 2048 * 2048

    local_rank = np.empty(M, dtype=np.int64)
    for k in range(NCORES):
        local_rank[ms[k]] = np.arange(c_ks[k])

    # dst[k, local_token, slot] = compact destination row on core k (or OOB sentinel)
    dst = np.full((NCORES, TPC, J), c_max, dtype=np.int32)
    dst[core_of, token_idx % TPC, slot] = local_rank
    return token_idx, scale, dst, ms, c_ks, c_max, J


def kernel(hidden_states, probs, routing_map):
    from concourse.bass_utils import run_bass_kernel_spmd

    hs = np.ascontiguousarray(np.asarray(hidden_states, dtype=np.float32).reshape(N, H))
    probs = np.asarray(probs, dtype=np.float32)
    rmap = np.asarray(routing_map).astype(bool)

    token_idx, scale, dst, ms, c_ks, c_max, J = _plan(probs, rmap)

    key = (J, c_max)
    if key not in _program_cache:
        _program_cache[key] = _build_program(J, c_max)
    nc = _program_cache[key]

    in_maps = []
    for k in range(NCORES):
        # pack per-tile metadata partition-major so each load is one contiguous DMA
        dstp = np.ascontiguousarray(
            dst[k].reshape(TILES, P, J).transpose(1, 0, 2).reshape(P, TILES * J)
        )
        scp = np.ascontiguousarray(scale[k * TPC:(k + 1) * TPC].reshape(TILES, P).T)
        in_maps.append(
            {
                "hs_chunk": hs[k * TPC:(k + 1) * TPC],
                "dst_idx": dstp,
                "scale_in": scp,
            }
        )

    results = run_bass_kernel_spmd(nc, in_maps, list(range(NCORES))).results

    output_total = np.concatenate(
        [results[k]["out_scaled"] for k in range(NCORES)], axis=0
    ).reshape(S, B, H)
    permuted = np.empty((M, H), dtype=np.float32)
    for k in range(NCORES):
        permuted[ms[k]] = results[k]["perm_part"][: c_ks[k]]
    tokens_per_expert = rmap.sum(axis=0).astype(np.int32)

    return output_total, tokens_per_expert, permuted


# revision 11
# speedup vs baseline: 1.0353x; 1.0353x over previous
"""MoE AllGather token dispatcher (permute -> identity experts -> unpermute).

Strategy: shard tokens contiguously across the 8 cores (2048 tokens each).
Since the "experts" are identity, output_total[t] = hs[t] * sum(routed probs of t),
so each core only needs its own token rows:
  - read each of its token rows once (contiguous 1MB tiles),
  - indirect-DMA-scatter each row to its permuted positions (renumbered into a
    compact per-core buffer; host reassembles by global position),
  - multiply by the per-token prob sum and store its slice of output_total.
Per-core HBM traffic = 16MB read + 48MB write = I/O minimum for this op.
"""

import sys
import numpy as np

for _p in ("/opt/trn_rl_repo", "/root/.axon_site/_ro/trn_rl_repo"):
    if _p not in sys.path:
        sys.path.append(_p)

S, B, H = 4096, 4, 2048
E = 64
TOPK = 2
N = S * B               # 16384 tokens
M = N * TOPK            # 32768 permuted rows
NCORES = 8
TPC = N // NCORES       # 2048 tokens per core
P = 128
TILES = TPC // P        # 16 tiles of 128 tokens per core

_program_cache = {}


def _build_program(J, c_max, iters=1, swdge_queues=1, wide=1, bufs=4):
    """SPMD program: identical on all cores; per-core data differs only in inputs.

    iters > 1 repeats the whole pipeline (for slope-based timing only).
    swdge_queues: spread indirect scatters round-robin over this many SWDGE queues.
    wide: how many 128-token tiles to move per load/store DMA.
    """
    import concourse.bacc as bacc
    import concourse.bass as bass
    import concourse.mybir as mybir
    import concourse.tile as tile

    nc = bacc.Bacc(num_swdge_queues=swdge_queues)
    hs_chunk = nc.dram_tensor("hs_chunk", [TPC, H], mybir.dt.float32, kind="ExternalInput")
    dst_idx = nc.dram_tensor("dst_idx", [P, TILES * J], mybir.dt.int32, kind="ExternalInput")
    scale_in = nc.dram_tensor("scale_in", [P, TILES], mybir.dt.float32, kind="ExternalInput")
    perm_part = nc.dram_tensor("perm_part", [c_max, H], mybir.dt.float32, kind="ExternalOutput")
    out_scaled = nc.dram_tensor("out_scaled", [TPC, H], mybir.dt.float32, kind="ExternalOutput")

    W = wide
    assert TILES % W == 0
    scat_n = 0
    with tile.TileContext(nc) as tc:
        with (
            tc.tile_pool(name="meta", bufs=1) as meta_pool,
            tc.tile_pool(name="io", bufs=bufs) as io_pool,
            tc.tile_pool(name="res", bufs=bufs) as res_pool,
        ):
            dst_t = meta_pool.tile([P, TILES * J], mybir.dt.int32)
            scale_t = meta_pool.tile([P, TILES], mybir.dt.float32)
            nc.sync.dma_start(out=dst_t[:], in_=dst_idx[:])
            nc.sync.dma_start(out=scale_t[:], in_=scale_in[:])
            for _it in range(iters):
                for g in range(TILES // W):
                    hs_t = io_pool.tile([P, W * H], mybir.dt.float32)
                    src = hs_chunk[g * W * P:(g + 1) * W * P, :]
                    if W > 1:
                        src = src.rearrange("(w p) h -> p w h", p=P)
                        nc.sync.dma_start(
                            out=hs_t[:].rearrange("p (w h) -> p w h", w=W), in_=src
                        )
                    else:
                        nc.sync.dma_start(out=hs_t[:], in_=src)
                    sc_t = res_pool.tile([P, W * H], mybir.dt.float32)
                    for w in range(W):
                        i = g * W + w
                        for j in range(J):
                            col = i * J + j
                            inst = nc.gpsimd.indirect_dma_start(
                                out=perm_part[:],
                                out_offset=bass.IndirectOffsetOnAxis(
                                    ap=dst_t[:, col:col + 1], axis=0
                                ),
                                in_=hs_t[:, w * H:(w + 1) * H],
                                in_offset=None,
                                bounds_check=c_max - 1,
                                oob_is_err=False,
                            )
                            if swdge_queues > 1:
                                q = scat_n % swdge_queues
                                inst.ins.queue = f"qPoolDynamic{q or ''}"
                                scat_n += 1
                        nc.vector.tensor_scalar_mul(
                            sc_t[:, w * H:(w + 1) * H],
                            hs_t[:, w * H:(w + 1) * H],
                            scale_t[:, i:i + 1],
                        )
                    dstap = out_scaled[g * W * P:(g + 1) * W * P, :]
                    if W > 1:
                        dstap = dstap.rearrange("(w p) h -> p w h", p=P)
                        nc.scalar.dma_start(
                            out=dstap, in_=sc_t[:].rearrange("p (w h) -> p w h", w=W)
                        )
                    else:
                        nc.scalar.dma_start(out=dstap, in_=sc_t[:])

    nc.compile()
    return nc


def _plan(probs, routing_map):
    """Reproduce the reference's permutation order and build per-core scatter plans."""
    maskT = np.ascontiguousarray(routing_map.T).reshape(-1)       # [E*N]
    true_pos = np.flatnonzero(maskT)
    if true_pos.size >= M:
        sorted_idx = true_pos[:M]
    else:
        false_pos = np.flatnonzero(~maskT)
        sorted_idx = np.concatenate([true_pos, false_pos[: M - true_pos.size]])
    token_idx = sorted_idx % N                                    # [M] source token per row
    permuted_probs = np.ascontiguousarray(probs.T).reshape(-1)[sorted_idx]

    # per-token sum of routed probs (accumulated in f64, stored f32)
    scale = np.bincount(token_idx, weights=permuted_probs.astype(np.float64), minlength=N)
    scale = scale.astype(np.float32)

    # occurrence slot of each permuted row within its token (rows of one token
    # appear in increasing m; stable argsort preserves that order)
    order = np.argsort(token_idx, kind="stable")
    sorted_tok = token_idx[order]
    is_new = np.empty(M, dtype=bool)
    is_new[0] = True
    np.not_equal(sorted_tok[1:], sorted_tok[:-1], out=is_new[1:])
    group_id = np.cumsum(is_new) - 1
    group_start = np.flatnonzero(is_new)
    slot = np.empty(M, dtype=np.int64)
    slot[order] = np.arange(M) - group_start[group_id]
    J = int(slot.max()) + 1

    core_of = token_idx // TPC
    ms = [np.flatnonzero(core_of == k) for k in range(NCORES)]
    c_ks = [int(x.size) for x in ms]
    # round the compact buffer size up to a multiple of 2048 so the compiled
    # program (and the on-disk NEFF cache key) is stable across inputs
    c_max = max(max(c_ks), 1)
    c_max = (c_max + 2047) // 2048 * 2048

    local_rank = np.empty(M, dtype=np.int64)
    for k in range(NCORES):
        local_rank[ms[k]] = np.arange(c_ks[k])

    # dst[k, local_token, slot] = compact destination row on core k (or OOB sentinel)
    dst = np.full((NCORES, TPC, J), c_max, dtype=np.int32)
    dst[core_of, token_idx % TPC, slot] = local_rank
    return token_idx, scale, dst, ms, c_ks, c_max, J


def kernel(hidden_states, probs, routing_map):
    from concourse.bass_utils import run_bass_kernel_spmd

    hs = np.ascontiguousarray(np.asarray(hidden_states, dtype=np.float32).reshape(N, H))
    probs = np.asarray(probs, dtype=np.float32)
    rmap = np.asarray(routing_map).astype(bool)

    token_idx, scale, dst, ms, c_ks, c_max, J = _plan(probs, rmap)

    key = (J, c_max)
    if key not in _program_cache:
        _program_cache[key] = _build_program(J, c_max)
    nc = _program_cache[key]

    in_maps = []
    for k in range(NCORES):
        # pack per-tile metadata partition-major so each load is one contiguous DMA
        dstp = np.ascontiguousarray(
            dst[k].reshape(TILES, P, J).transpose(1, 0, 2).reshape(P, TILES * J)
        )
        scp = np.ascontiguousarray(scale[k * TPC:(k + 1) * TPC].reshape(TILES, P).T)
        in_maps.append(
            {
                "hs_chunk": hs[k * TPC:(k + 1) * TPC],
                "dst_idx": dstp,
                "scale_in": scp,
            }
        )

    # the shared terminal occasionally reports a transient wedged device
    # (NRT_EXEC_UNIT_UNRECOVERABLE); it recovers on its own, so retry those
    results = None
    for attempt in range(4):
        try:
            results = run_bass_kernel_spmd(nc, in_maps, list(range(NCORES))).results
            break
        except Exception as e:
            transient = any(
                s in (type(e).__name__ + str(e))
                for s in ("UNAVAILABLE", "NRT", "unrecoverable", "PassThrough")
            )
            if attempt == 3 or not transient:
                raise
            import time as _time
            _time.sleep(20.0 * (attempt + 1))

    output_total = np.concatenate(
        [results[k]["out_scaled"] for k in range(NCORES)], axis=0
    ).reshape(S, B, H)
    permuted = np.empty((M, H), dtype=np.float32)
    for k in range(NCORES):
        permuted[ms[k]] = results[k]["perm_part"][: c_ks[k]]
    tokens_per_expert = rmap.sum(axis=0).astype(np.int32)

    return output_total, tokens_per_expert, permuted


# revision 18
# speedup vs baseline: 1.6652x; 1.6084x over previous
"""MoE AllGather token dispatcher (permute -> identity experts -> unpermute).

Strategy: shard tokens contiguously across the 8 cores (2048 tokens each).
Since the "experts" are identity, output_total[t] = hs[t] * sum(routed probs of t),
so each core only needs its own token rows:
  - read each of its token rows once (contiguous 1MB tiles),
  - indirect-DMA-scatter each row to its permuted positions (renumbered into a
    compact per-core buffer; host reassembles by global position),
  - multiply by the per-token prob sum and store its slice of output_total.
Per-core HBM traffic = 16MB read + 48MB write = I/O minimum for this op.
"""

import sys
import numpy as np

for _p in ("/opt/trn_rl_repo", "/root/.axon_site/_ro/trn_rl_repo"):
    if _p not in sys.path:
        sys.path.append(_p)

S, B, H = 4096, 4, 2048
E = 64
TOPK = 2
N = S * B               # 16384 tokens
M = N * TOPK            # 32768 permuted rows
NCORES = 8
TPC = N // NCORES       # 2048 tokens per core
P = 128
TILES = TPC // P        # 16 tiles of 128 tokens per core

_program_cache = {}


def _group_of(tiles):
    for g in (4, 2):
        if tiles % g == 0:
            return g
    return 1


def _build_program(J, c_max, iters=1, swdge_queues=1, wide=1, bufs=4, group=1,
                   use_bounds=True):
    """SPMD program: identical on all cores; per-core data differs only in inputs.

    iters > 1 repeats the whole pipeline (for slope-based timing only).
    swdge_queues: spread indirect scatters round-robin over this many SWDGE queues.
    wide: how many 128-token tiles to move per load/store DMA.
    group: batch G tiles per load/store AND per indirect scatter ([128, G] index
      slice against a [128, G*H] source tile; index (p, w) pairs with source
      chunk p*G + w = partition p, block w — HW pairing verified by probe).
    """
    import concourse.bacc as bacc
    import concourse.bass as bass
    import concourse.mybir as mybir
    import concourse.tile as tile

    if group > 1:
        assert wide == 1
        return _build_program_grouped(J, c_max, iters, group, bufs)

    nc = bacc.Bacc(num_swdge_queues=swdge_queues)
    hs_chunk = nc.dram_tensor("hs_chunk", [TPC, H], mybir.dt.float32, kind="ExternalInput")
    dst_idx = nc.dram_tensor("dst_idx", [P, TILES * J], mybir.dt.int32, kind="ExternalInput")
    scale_in = nc.dram_tensor("scale_in", [P, TILES], mybir.dt.float32, kind="ExternalInput")
    perm_part = nc.dram_tensor("perm_part", [c_max, H], mybir.dt.float32, kind="ExternalOutput")
    out_scaled = nc.dram_tensor("out_scaled", [TPC, H], mybir.dt.float32, kind="ExternalOutput")

    W = wide
    assert TILES % W == 0
    scat_n = 0
    with tile.TileContext(nc) as tc:
        with (
            tc.tile_pool(name="meta", bufs=1) as meta_pool,
            tc.tile_pool(name="io", bufs=bufs) as io_pool,
            tc.tile_pool(name="res", bufs=bufs) as res_pool,
        ):
            dst_t = meta_pool.tile([P, TILES * J], mybir.dt.int32)
            scale_t = meta_pool.tile([P, TILES], mybir.dt.float32)
            nc.sync.dma_start(out=dst_t[:], in_=dst_idx[:])
            nc.sync.dma_start(out=scale_t[:], in_=scale_in[:])
            for _it in range(iters):
                for g in range(TILES // W):
                    hs_t = io_pool.tile([P, W * H], mybir.dt.float32)
                    src = hs_chunk[g * W * P:(g + 1) * W * P, :]
                    if W > 1:
                        src = src.rearrange("(w p) h -> p w h", p=P)
                        nc.sync.dma_start(
                            out=hs_t[:].rearrange("p (w h) -> p w h", w=W), in_=src
                        )
                    else:
                        nc.sync.dma_start(out=hs_t[:], in_=src)
                    sc_t = res_pool.tile([P, W * H], mybir.dt.float32)
                    for w in range(W):
                        i = g * W + w
                        for j in range(J):
                            col = i * J + j
                            # bounds_check only when sentinel (skip) slots exist;
                            # without it the Q7 emission loop does less work per row
                            inst = nc.gpsimd.indirect_dma_start(
                                out=perm_part[:],
                                out_offset=bass.IndirectOffsetOnAxis(
                                    ap=dst_t[:, col:col + 1], axis=0
                                ),
                                in_=hs_t[:, w * H:(w + 1) * H],
                                in_offset=None,
                                bounds_check=c_max - 1 if use_bounds else None,
                                oob_is_err=False if use_bounds else True,
                            )
                            if swdge_queues > 1:
                                q = scat_n % swdge_queues
                                inst.ins.queue = f"qPoolDynamic{q or ''}"
                                scat_n += 1
                        nc.vector.tensor_scalar_mul(
                            sc_t[:, w * H:(w + 1) * H],
                            hs_t[:, w * H:(w + 1) * H],
                            scale_t[:, i:i + 1],
                        )
                    dstap = out_scaled[g * W * P:(g + 1) * W * P, :]
                    if W > 1:
                        dstap = dstap.rearrange("(w p) h -> p w h", p=P)
                        nc.scalar.dma_start(
                            out=dstap, in_=sc_t[:].rearrange("p (w h) -> p w h", w=W)
                        )
                    else:
                        nc.scalar.dma_start(out=dstap, in_=sc_t[:])

    nc.compile()
    return nc


def _build_program_grouped(J, c_max, iters, G, bufs):
    """Grouped variant: G tiles per DMA; one [128, G]-index scatter per (group, slot)."""
    import concourse.bacc as bacc
    import concourse.bass as bass
    import concourse.mybir as mybir
    import concourse.tile as tile

    nc = bacc.Bacc()
    hs_chunk = nc.dram_tensor("hs_chunk", [TPC, H], mybir.dt.float32, kind="ExternalInput")
    dst_idx = nc.dram_tensor("dst_idx", [P, TILES * J], mybir.dt.int32, kind="ExternalInput")
    scale_in = nc.dram_tensor("scale_in", [P, TILES], mybir.dt.float32, kind="ExternalInput")
    perm_part = nc.dram_tensor("perm_part", [c_max, H], mybir.dt.float32, kind="ExternalOutput")
    out_scaled = nc.dram_tensor("out_scaled", [TPC, H], mybir.dt.float32, kind="ExternalOutput")

    # two pools of [P, G*H] f32 tiles must fit in ~190KB/partition SBUF
    bufs = min(bufs, max(2, 8 // G))
    NG = TILES // G
    with tile.TileContext(nc) as tc:
        with (
            tc.tile_pool(name="meta", bufs=1) as meta_pool,
            tc.tile_pool(name="io", bufs=bufs) as io_pool,
            tc.tile_pool(name="res", bufs=bufs) as res_pool,
        ):
            # meta layout (grouped): dst_t[p, g*(J*G) + j*G + w] = dst of
            # (tile g*G+w, token p, slot j); scale_t[p, i] per tile as before
            dst_t = meta_pool.tile([P, TILES * J], mybir.dt.int32)
            scale_t = meta_pool.tile([P, TILES], mybir.dt.float32)
            nc.sync.dma_start(out=dst_t[:], in_=dst_idx[:])
            nc.sync.dma_start(out=scale_t[:], in_=scale_in[:])
            for _it in range(iters):
                for g in range(NG):
                    hs_t = io_pool.tile([P, G * H], mybir.dt.float32)
                    src = hs_chunk[g * G * P:(g + 1) * G * P, :]
                    nc.sync.dma_start(
                        out=hs_t[:].rearrange("p (w h) -> p w h", w=G),
                        in_=src.rearrange("(w p) h -> p w h", p=P),
                    )
                    for j in range(J):
                        c0 = g * (J * G) + j * G
                        nc.gpsimd.indirect_dma_start(
                            out=perm_part[:],
                            out_offset=bass.IndirectOffsetOnAxis(
                                ap=dst_t[:, c0:c0 + G], axis=0
                            ),
                            in_=hs_t[:],
                            in_offset=None,
                            bounds_check=c_max - 1,
                            oob_is_err=False,
                        )
                    sc_t = res_pool.tile([P, G * H], mybir.dt.float32)
                    for w in range(G):
                        i = g * G + w
                        nc.vector.tensor_scalar_mul(
                            sc_t[:, w * H:(w + 1) * H],
                            hs_t[:, w * H:(w + 1) * H],
                            scale_t[:, i:i + 1],
                        )
                    nc.scalar.dma_start(
                        out=out_scaled[g * G * P:(g + 1) * G * P, :].rearrange(
                            "(w p) h -> p w h", p=P
                        ),
                        in_=sc_t[:].rearrange("p (w h) -> p w h", w=G),
                    )

    nc.compile()
    return nc


def _pack_inmaps(hs, scale, dst, J, G):
    """Per-core input dicts; dst packed for the given group size."""
    in_maps = []
    for k in range(NCORES):
        if G > 1:
            # [TILES, P, J] -> [NG, G, P, J] -> [P, NG, J, G] -> [P, TILES*J]
            dstp = np.ascontiguousarray(
                dst[k].reshape(TILES // G, G, P, J)
                .transpose(2, 0, 3, 1)
                .reshape(P, TILES * J)
            )
        else:
            dstp = np.ascontiguousarray(
                dst[k].reshape(TILES, P, J).transpose(1, 0, 2).reshape(P, TILES * J)
            )
        scp = np.ascontiguousarray(scale[k * TPC:(k + 1) * TPC].reshape(TILES, P).T)
        in_maps.append(
            {"hs_chunk": hs[k * TPC:(k + 1) * TPC], "dst_idx": dstp, "scale_in": scp}
        )
    return in_maps


def _plan(probs, routing_map):
    """Reproduce the reference's permutation order and build per-core scatter plans."""
    maskT = np.ascontiguousarray(routing_map.T).reshape(-1)       # [E*N]
    true_pos = np.flatnonzero(maskT)
    if true_pos.size >= M:
        sorted_idx = true_pos[:M]
    else:
        false_pos = np.flatnonzero(~maskT)
        sorted_idx = np.concatenate([true_pos, false_pos[: M - true_pos.size]])
    token_idx = sorted_idx % N                                    # [M] source token per row
    permuted_probs = np.ascontiguousarray(probs.T).reshape(-1)[sorted_idx]

    # per-token sum of routed probs (accumulated in f64, stored f32)
    scale = np.bincount(token_idx, weights=permuted_probs.astype(np.float64), minlength=N)
    scale = scale.astype(np.float32)

    # occurrence slot of each permuted row within its token (rows of one token
    # appear in increasing m; stable argsort preserves that order)
    order = np.argsort(token_idx, kind="stable")
    sorted_tok = token_idx[order]
    is_new = np.empty(M, dtype=bool)
    is_new[0] = True
    np.not_equal(sorted_tok[1:], sorted_tok[:-1], out=is_new[1:])
    group_id = np.cumsum(is_new) - 1
    group_start = np.flatnonzero(is_new)
    slot = np.empty(M, dtype=np.int64)
    slot[order] = np.arange(M) - group_start[group_id]
    J = int(slot.max()) + 1

    core_of = token_idx // TPC
    ms = [np.flatnonzero(core_of == k) for k in range(NCORES)]
    c_ks = [int(x.size) for x in ms]
    # round the compact buffer size up to a multiple of 2048 so the compiled
    # program (and the on-disk NEFF cache key) is stable across inputs
    c_max = max(max(c_ks), 1)
    c_max = (c_max + 2047) // 2048 * 2048

    local_rank = np.empty(M, dtype=np.int64)
    for k in range(NCORES):
        local_rank[ms[k]] = np.arange(c_ks[k])

    # dst[k, local_token, slot] = compact destination row on core k (or OOB sentinel)
    dst = np.full((NCORES, TPC, J), c_max, dtype=np.int32)
    dst[core_of, token_idx % TPC, slot] = local_rank
    return token_idx, scale, dst, ms, c_ks, c_max, J


def kernel(hidden_states, probs, routing_map):
    from concourse.bass_utils import run_bass_kernel_spmd

    hs = np.ascontiguousarray(np.asarray(hidden_states, dtype=np.float32).reshape(N, H))
    probs = np.asarray(probs, dtype=np.float32)
    rmap = np.asarray(routing_map).astype(bool)

    token_idx, scale, dst, ms, c_ks, c_max, J = _plan(probs, rmap)

    # note: dropping bounds_check when no sentinel slots exist was measured
    # exact but perf-neutral, so keep the always-bounds program (warm NEFF cache)
    key = (J, c_max)
    if key not in _program_cache:
        _program_cache[key] = _build_program(J, c_max)
    nc = _program_cache[key]

    in_maps = []
    for k in range(NCORES):
        # pack per-tile metadata partition-major so each load is one contiguous DMA
        dstp = np.ascontiguousarray(
            dst[k].reshape(TILES, P, J).transpose(1, 0, 2).reshape(P, TILES * J)
        )
        scp = np.ascontiguousarray(scale[k * TPC:(k + 1) * TPC].reshape(TILES, P).T)
        in_maps.append(
            {
                "hs_chunk": hs[k * TPC:(k + 1) * TPC],
                "dst_idx": dstp,
                "scale_in": scp,
            }
        )

    # the shared terminal occasionally reports a transient wedged device
    # (NRT_EXEC_UNIT_UNRECOVERABLE); it recovers on its own, so retry those
    results = None
    for attempt in range(4):
        try:
            results = run_bass_kernel_spmd(nc, in_maps, list(range(NCORES))).results
            break
        except Exception as e:
            transient = any(
                s in (type(e).__name__ + str(e))
                for s in ("UNAVAILABLE", "NRT", "unrecoverable", "PassThrough")
            )
            if attempt == 3 or not transient:
                raise
            import time as _time
            _time.sleep(20.0 * (attempt + 1))

    output_total = np.concatenate(
        [results[k]["out_scaled"] for k in range(NCORES)], axis=0
    ).reshape(S, B, H)
    permuted = np.empty((M, H), dtype=np.float32)
    for k in range(NCORES):
        permuted[ms[k]] = results[k]["perm_part"][: c_ks[k]]
    tokens_per_expert = rmap.sum(axis=0).astype(np.int32)

    return output_total, tokens_per_expert, permuted
